# revision 25
# baseline (speedup 1.0000x reference)
"""Causal self-attention (GPT-style block) on 8 Trainium2 NeuronCores.

Sharding: tensor-parallel over heads. 16 heads / 8 cores = 2 heads per core.
- c_attn column-parallel: each core computes q/k/v for its 2 heads from the
  full input x (streamed to every core).
- attention: fully local per core (its 2 heads, all 4 batches).
- c_proj token-parallel after an AllToAll of the attention output; each core
  returns fully-reduced output rows for its own token shard. The final
  half-batch skips the exchange (row-parallel partials summed on the host)
  so the tail never waits on a collective.

Implementation notes (all matmul operands bf16; rel-err budget is 2e-2):
- q,k are produced channel-major ([chan, tok]); v is produced TOKEN-major
  directly in stage 1 (stationary = x-tile, moving = w_v), with the v-bias
  and the softmax-denominator ones-column folded into a single K=1 matmul
  (moving row bva has bias values + 1.0 at the ones slots).
- Scores are computed transposed: S^T[key, query] with the 2 heads packed
  into the two 64-row halves of the PE array (tile_position row tiling).
- exp on ACT (scale=1/8) with one instruction per key tile (strided AP
  covers both heads); causal mask applied multiplicatively on ep (DVE).
- PV is computed in the [query, chan] orientation: stationary = ep tile
  [keys, 128 queries], moving = vaug [keys, 65] -> full 128-partition PE
  utilization, and the softmax denominator lands in psum column 64 as a
  per-partition scalar. Normalization is a single fused DVE multiply
  (psum * broadcast(1/l)) that also evicts to SBUF bf16.
- y (token-major) is exchanged per-batch via 3 AllToAlls + 1 single-unit
  AllToAll; receivers PE-transpose the 8 received [tok, chan] tiles to
  channel-major and run c_proj with the full w_proj.
- Emission is software-pipelined: stage-1 of block g+1 and ready
  projection units are interleaved ("filler chunks") inside the attention
  j-loops so the PE instruction stream never head-blocks on the ACT exp.
"""

import numpy as np

P = 128
B = 4
T = 2048
BT = B * T            # 8192 tokens
C = 1024
KT = C // P           # 8 contraction tiles of 128 input channels
NTB = BT // 512       # 16 token blocks of 512
HD = 64               # head dim
NQ = T // 512         # 4 query blocks per batch
NCORES = 8

_CACHED = {}


def _build_nc():
    import concourse.mybir as mybir
    import concourse.tile as tile
    from concourse import bacc
    from concourse.masks import make_identity

    f32 = mybir.dt.float32
    bf16 = mybir.dt.bfloat16
    EXP = mybir.ActivationFunctionType.Exp

    nc = bacc.Bacc("TRN2", target_bir_lowering=False, debug=False,
                   num_devices=NCORES)

    xp = nc.dram_tensor("xp", [NTB, P, KT, 512], bf16, kind="ExternalInput")
    wq = nc.dram_tensor("wq", [P, KT, P], bf16, kind="ExternalInput")
    wk = nc.dram_tensor("wk", [P, KT, P], bf16, kind="ExternalInput")
    wva = nc.dram_tensor("wva", [P, KT, P], bf16, kind="ExternalInput")
    wp = nc.dram_tensor("wp", [P, KT, C], bf16, kind="ExternalInput")
    wpr = nc.dram_tensor("wpr", [P, C], bf16, kind="ExternalInput")
    bq = nc.dram_tensor("bq", [P, 1], f32, kind="ExternalInput")
    bk = nc.dram_tensor("bk", [P, 1], f32, kind="ExternalInput")
    # exchange units 0..5: my 128 tokens x all 1024 channels, projected
    yp = nc.dram_tensor("yp", [6, P, C], bf16, kind="ExternalOutput")
    # units 6,7 (batch 3): row-parallel partials, host sums over cores
    ypl = nc.dram_tensor("ypl", [T, C], bf16, kind="ExternalOutput")

    with tile.TileContext(nc) as tc:
        with (
            tc.tile_pool(name="const", bufs=1) as const,
            tc.tile_pool(name="xt", bufs=2) as xt_pool,
            tc.tile_pool(name="slab", bufs=2) as slab_pool,
            tc.tile_pool(name="e", bufs=18) as e_pool,
            tc.tile_pool(name="r", bufs=4) as r_pool,
            tc.tile_pool(name="y", bufs=16) as y_pool,
            tc.tile_pool(name="ygt", bufs=2) as ygt_pool,
            tc.tile_pool(name="yg", bufs=2) as yg_pool,
            tc.tile_pool(name="ob", bufs=3) as ob_pool,
            tc.tile_pool(name="dram", bufs=1, space="DRAM") as dram_pool,
            tc.tile_pool(name="ps1", bufs=2, space="PSUM") as ps1_pool,
            tc.tile_pool(name="pss", bufs=2, space="PSUM") as pss_pool,
            tc.tile_pool(name="po", bufs=2, space="PSUM") as po_pool,
        ):
            # --- DRAM staging for the exchanges ---
            # per-batch AllToAll: dest core gets [2 units, 128 tok, 128 chan]
            g_in = [dram_pool.tile([NCORES, 2, P, P], bf16, name=f"g_in{b}",
                                   tag=f"g_in{b}") for b in range(3)]
            g_out = [dram_pool.tile([NCORES, 2, P, P], bf16, name=f"g_out{b}",
                                    tag=f"g_out{b}") for b in range(3)]

            # --- constants / weights resident in SBUF ---
            wq_sb = const.tile([P, KT, P], bf16)
            wk_sb = const.tile([P, KT, P], bf16)
            wva_sb = const.tile([P, KT, P], bf16)
            wp_sb = const.tile([P, KT, C], bf16)
            wpr_sb = const.tile([P, C], bf16)
            bq_sb = const.tile([P, 1], f32)
            bk_sb = const.tile([P, 1], f32)
            nc.sync.dma_start(wq_sb[:, 0], wq[:, 0])
            nc.sync.dma_start(wq_sb[:, 1:KT], wq[:, 1:KT])

            ident = const.tile([P, P], bf16)


            make_identity(nc, ident[:])
            # PE warmup: dummy matmuls on ident while the first DMAs land
            # ramp the p-state so real work starts at full clock
            wtmp = const.tile([P, P], bf16)
            nc.vector.memset(wtmp[:], 0.5)

            def warmup():
                pw = ps1_pool.tile([P, P], bf16, tag="ps1", name="warm")
                for _ in range(16):
                    nc.tensor.transpose(pw[:], wtmp[:], wtmp[:])

            # mask[p, s] = 1.0 if s >= p else 0.0 (keep upper-right triangle)
            mask_f = const.tile([P, P], f32)
            nc.gpsimd.memset(mask_f[:], 1.0)
            nc.gpsimd.affine_select(
                out=mask_f[:],
                in_=mask_f[:],
                compare_op=mybir.AluOpType.is_ge,
                fill=0.0,
                base=0,
                pattern=[[1, P]],
                channel_multiplier=-1,
            )
            mask_sb = const.tile([P, P], bf16)
            nc.vector.tensor_copy(mask_sb[:], mask_f[:])

            # ---------------- stage 1 (qkv) chunk emitters ----------------
            xt_tiles = {}

            def load_xt(g):
                xt_tiles[g] = xt_pool.tile([P, KT, 512], bf16, name=f"xt{g}", tag="xt")
                nc.sync.dma_start(xt_tiles[g][:], xp[g])

            # per-batch slabs, double-buffered (batch b and b+1 in flight)
            qT_t = {}
            kT_t = {}
            va_t = {}

            def get_slabs(b):
                if b not in qT_t:
                    qT_t[b] = slab_pool.tile([P, T], bf16, tag="qT",
                                             name=f"qT{b}")
                    kT_t[b] = slab_pool.tile([P, T], bf16, tag="kT",
                                             name=f"kT{b}")
                    va_t[b] = slab_pool.tile([P, 16, 2, 65], bf16, tag="va",
                                             name=f"va{b}")
                    nc.vector.memset(va_t[b][:, :, :, 64], 1.0)
                return qT_t[b], kT_t[b], va_t[b]

            def s1_chunks(b, lb):
                """Chunks for stage-1 of 512-token block lb of batch b."""
                g = b * NQ + lb
                qT, kT, vaug = get_slabs(b)
                sl = slice(lb * 512, (lb + 1) * 512)

                def qk(w_sb, b_sb, dst):
                    def run(w_sb=w_sb, b_sb=b_sb, dst=dst):
                        if g + 1 < NTB:
                            if g + 1 not in xt_tiles:
                                load_xt(g + 1)
                        xt = xt_tiles[g]
                        ps = ps1_pool.tile([P, 512], f32, tag="ps1", name="ps")
                        for kt in range(KT):
                            nc.tensor.matmul(ps[:], w_sb[:, kt, :],
                                             xt[:, kt, :],
                                             start=(kt == 0),
                                             stop=(kt == KT - 1))
                        nc.vector.tensor_scalar_add(dst[:, sl], ps[:], b_sb[:])
                    return run

                def vpair(half):
                    def run(half=half):
                        xt = xt_tiles[g]
                        psv = ps1_pool.tile([P, 2, 2, 64], f32, tag="ps1", name="psv")
                        for ts2 in range(2):
                            ts = half * 2 + ts2
                            tsl = slice(ts * P, (ts + 1) * P)
                            for kt in range(KT):
                                nc.tensor.matmul(psv[:, ts2], xt[:, kt, tsl],
                                                 wva_sb[:, kt],
                                                 start=(kt == 0),
                                                 stop=(kt == KT - 1))
                        jt = lb * 4 + half * 2
                        nc.vector.tensor_copy(vaug[:, jt:jt + 2, :, 0:64],
                                              psv[:])
                    return run

                cks = [qk(wq_sb, bq_sb, qT), qk(wk_sb, bk_sb, kT),
                       vpair(0), vpair(1)]
                for ck, cost in zip(cks, (1900, 1800, 1100, 1100)):
                    ck.cost = cost
                return cks

            # ---------------- projection chunk emitters ----------------
            wp_loaded = []

            def load_wp():
                if not wp_loaded:
                    nc.sync.dma_start(wp_sb[:], wp[:])
                    nc.sync.dma_start(wpr_sb[:], wpr[:])
                    wp_loaded.append(True)

            def proj_chunks(u):
                """Project exchange unit u (0..6): my 128 tokens x 1024 ch."""
                state = {}

                def load_tr(half):
                    def run(half=half):
                        if half == 0:
                            yg = yg_pool.tile([P, NCORES, P], bf16, tag="yg", name="yg")
                            nc.sync.dma_start(
                                yg[:],
                                g_out[u // 2][:, u % 2].rearrange(
                                    "s p t -> p s t"))
                            state["yg"] = yg
                            state["ygT"] = ygt_pool.tile([P, NCORES, P], bf16,
                                                         tag="ygt", name="ygT")
                        yg, ygT = state["yg"], state["ygT"]
                        pst = po_pool.tile([P, 512], bf16, tag="po",
                                           name="pst")
                        for s in range(4):
                            src = half * 4 + s
                            nc.tensor.transpose(pst[:, s * P:(s + 1) * P],
                                                yg[:, src, :], ident[:])
                        nc.vector.tensor_copy(
                            ygT[:, half * 4:(half + 1) * 4], pst[:])
                    return run

                def mm(half):
                    def run(half=half):
                        ygT = state["ygT"]
                        pp = ps1_pool.tile([P, 512], f32, tag="ps1", name="pp")
                        for ct in range(KT):
                            nc.tensor.matmul(
                                pp[:], ygT[:, ct, :],
                                wp_sb[:, ct, half * 512:(half + 1) * 512],
                                start=(ct == 0), stop=(ct == KT - 1))
                        if half == 0:
                            state["ob"] = ob_pool.tile([P, C], bf16, tag="ob", name="ob")
                        ob = state["ob"]
                        # DVE for both halves: ACT's in-order queue is
                        # exp-latency-critical in these windows
                        if half == 0:
                            nc.vector.tensor_copy(ob[:, 0:512], pp[:])
                        else:
                            nc.vector.tensor_copy(ob[:, 512:C], pp[:])
                            nc.sync.dma_start(yp[u], ob[:])
                    return run

                cks = [load_tr(0), load_tr(1), mm(0), mm(1)]
                for ck, cost in zip(cks, (400, 300, 1750, 1750)):
                    ck.cost = cost
                return cks

            def partial_chunks(u, tiles=range(8)):
                """Row-parallel partial projection of b3 half-batch u-6,
                pipelined: transposes run ahead of the matmuls so the PE
                never waits a full DVE eviction round-trip."""
                i0 = (u - 6) * 2
                yT = {}

                def tr(t):
                    def run(t=t):
                        ytile = y_tiles[(3, i0 + t // 4)][t % 4]
                        pst = ps1_pool.tile([P, P], bf16, tag="ps1",
                                            name="pstp")
                        nc.tensor.transpose(pst[:], ytile[:], ident[:])
                        yT[t] = ygt_pool.tile([P, P], bf16, tag="ygt",
                                              name="yTp")
                        nc.vector.tensor_copy(yT[t][:], pst[:])
                    return run

                def mm(t):
                    def run(t=t):
                        row = (u - 6) * 1024 + t * P
                        pp0 = ps1_pool.tile([P, 512], f32, tag="ps1",
                                            name="pp0")
                        nc.tensor.matmul(pp0[:], yT[t][:], wpr_sb[:, 0:512],
                                         start=True, stop=True)
                        ob = ob_pool.tile([P, C], bf16, tag="ob", name="obp")
                        nc.vector.tensor_copy(ob[:, 0:512], pp0[:])
                        pp1 = ps1_pool.tile([P, 512], f32, tag="ps1",
                                            name="pp1")
                        nc.tensor.matmul(pp1[:], yT[t][:], wpr_sb[:, 512:C],
                                         start=True, stop=True)
                        nc.scalar.copy(ob[:, 512:C], pp1[:])
                        nc.sync.dma_start(ypl[row:row + P, :], ob[:])
                    return run

                if tiles is None:
                    return tr, mm
                tl = list(tiles)
                out = [tr(t) for t in tl[:2]]
                for k, t in enumerate(tl):
                    if k + 2 < len(tl):
                        out.append(tr(tl[k + 2]))
                    out.append(mm(t))
                return out

            # ---------------- attention ----------------
            y_tiles = {}   # (b, i) -> [y tile per subtile gt2]

            def att_block(b, i, filler, post_gt=None):
                """Query block i of batch b, pumping filler chunks inside.

                Two phases per block: an S-phase (scores+exp+mask, all ep
                tiles kept resident) and a PV-phase where each (subtile,
                head) PSUM accumulation group runs CONSECUTIVELY — real HW
                corrupts interleaved open groups within one psum bank.
                """
                qT, kT, vaug = get_slabs(b)
                nj = 4 * (i + 1)
                qsl0 = i * 512

                def emit_s(j):
                    q0 = max(0, j - 4 * i) * P
                    jsl = slice(j * P, (j + 1) * P)
                    qsl = slice(qsl0 + q0, qsl0 + 512)
                    psp = pss_pool.tile([P, 2, 512], f32, tag="pss",
                                        name=f"psp{j % 2}")
                    nc.tensor.matmul(psp[:, 0, q0:512], kT[0:HD, jsl],
                                     qT[0:HD, qsl], start=True, stop=True,
                                     tile_position=(0, 0))
                    nc.tensor.matmul(psp[:, 1, q0:512], kT[HD:P, jsl],
                                     qT[HD:P, qsl], start=True, stop=True,
                                     tile_position=(HD, 0))
                    ep = e_pool.tile([P, 2, 512], bf16, tag="e",
                                     name=f"ep{j}")
                    nc.scalar.activation(ep[:, :, q0:512], psp[:, :, q0:512],
                                         EXP, scale=0.125)
                    if j - 4 * i >= 0:
                        msl = slice(q0, q0 + P)
                        nc.gpsimd.tensor_mul(
                            ep[:, :, msl], ep[:, :, msl],
                            mask_sb[:].unsqueeze(1).broadcast_to((P, 2, P)))
                    return ep

                # --- S-phase ---
                eps = {}
                fi = 0
                for j in range(nj):
                    eps[j] = emit_s(j)
                    while fi * nj < (j + 1) * len(filler):
                        filler[fi]()
                        fi += 1

                # --- PV-phase ---
                ytl = []
                for gt2 in range(4):
                    nd = 4 * i + gt2 + 1   # visible key tiles
                    po = po_pool.tile([P, 2, 65], f32, tag="po", name="po")
                    for h in range(2):
                        for j in range(nd):
                            nc.tensor.matmul(
                                po[:, h], eps[j][:, h, gt2 * P:(gt2 + 1) * P],
                                vaug[:, j, h, :], start=(j == 0),
                                stop=(j == nd - 1))
                    # fused normalize + eviction: y = po * broadcast(1/l)
                    ytile = y_pool.tile([P, 2, 64], bf16, tag="y", name="yt")
                    ytl.append(ytile)
                    r = r_pool.tile([P, 2], f32, tag="r", name="r")
                    nc.vector.reciprocal(r[:], po[:, :, 64])
                    nc.vector.tensor_mul(
                        ytile[:], po[:, :, 0:64],
                        r[:].unsqueeze(2).broadcast_to((P, 2, 64)))
                    u = 2 * b + i // 2
                    if u < 6:
                        # A2A staging (dest core = subtile index in half)
                        dst = (i % 2) * 4 + gt2
                        nc.sync.dma_start(g_in[b][dst, i // 2], ytile[:])
                    # u >= 6: consumed locally by partial_chunks
                    y_tiles[(b, i)] = ytl
                    if post_gt is not None:
                        for f in post_gt(gt2):
                            f()
                for f in filler[fi:]:
                    f()

            # ---------------- schedule ----------------
            def exchange(b):
                nc.gpsimd.collective_compute(
                    "AllToAll",
                    mybir.AluOpType.bypass,
                    replica_groups=[list(range(NCORES))],
                    ins=[g_in[b][:]],
                    outs=[g_out[b][:]],
                )

            # first x block arrives per-kt so the first q matmuls can
            # start as soon as wq + kt0 land
            xt_tiles[0] = xt_pool.tile([P, KT, 512], bf16, name="xt0",
                                       tag="xt")
            warmup()
            nc.sync.dma_start(xt_tiles[0][:, 0:4], xp[0][:, 0:4])
            nc.sync.dma_start(wk_sb[:], wk[:])
            nc.sync.dma_start(xt_tiles[0][:, 4:KT], xp[0][:, 4:KT])
            nc.sync.dma_start(bq_sb[:], bq[:])
            nc.sync.dma_start(bk_sb[:], bk[:])
            nc.sync.dma_start(wva_sb[:], wva[:])
            for f in s1_chunks(0, 0):
                f()

            # filler assignment per attention window (b, i)
            windows = {}
            for b in range(B):
                for i in range(NQ):
                    g = b * NQ + i + 1   # next stage-1 block, pipelined
                    windows[(b, i)] = s1_chunks(g // NQ, g % NQ) \
                        if g < NTB else []
            # projection units placed once their collective has finished
            PROJ_SLOT = {0: (2, 0), 1: (2, 1), 2: (2, 3), 3: (3, 0),
                         4: (3, 3), 5: (3, 3)}
            proj_lists = {}
            for u, slot in PROJ_SLOT.items():
                proj_lists.setdefault(slot, []).append(u)
            p6tr, p6mm = partial_chunks(6, None)
            p7tr, p7mm = partial_chunks(7, None)

            for b in range(B):
                for i in range(NQ):
                    filler = list(windows[(b, i)])
                    for u in proj_lists.get((b, i), []):
                        filler += proj_chunks(u)
                    post_gt = None
                    if (b, i) == (3, 2):
                        filler += partial_chunks(6)
                    if (b, i) == (3, 3):
                        filler += partial_chunks(7, range(4))

                        def post_gt(gt2):
                            # drain U7 tiles 4..7 inline with the PV-phase
                            out = [p7tr(4 + gt2)]
                            if gt2 > 0:
                                out.append(p7mm(3 + gt2))
                            return out
                    if (b, i) == (1, 2):
                        load_wp()
                    att_block(b, i, filler, post_gt)
                    if i == 3 and b < 3:
                        exchange(b)

            p7mm(7)()

    nc.compile()
    return nc


def _prep_inputs(x, w_attn, b_attn, w_proj):
    import ml_dtypes
    bf16 = ml_dtypes.bfloat16

    x = np.asarray(x, dtype=np.float32)
    w_attn = np.asarray(w_attn, dtype=np.float32)
    b_attn = np.asarray(b_attn, dtype=np.float32)
    w_proj = np.asarray(w_proj, dtype=np.float32)

    x_flat = x.reshape(BT, C)
    # xp[tb, p, kt, s] = x_flat[tb*512+s, kt*128+p]
    xp = np.ascontiguousarray(
        x_flat.T.reshape(KT, P, NTB, 512).transpose(2, 1, 0, 3)).astype(bf16)

    wp = np.ascontiguousarray(
        w_proj.reshape(KT, P, C).transpose(1, 0, 2)).astype(bf16)
    in_maps = []
    for c in range(NCORES):
        cols = slice(P * c, P * (c + 1))

        def wslice(off):
            w = w_attn[:, off + P * c: off + P * (c + 1)]   # [1024, 128]
            return np.ascontiguousarray(
                w.reshape(KT, P, P).transpose(1, 0, 2))

        wva = wslice(2 * C)                                  # [P, KT, 128]
        in_maps.append({
            "xp": xp,
            "wq": wslice(0).astype(bf16),
            "wk": wslice(C).astype(bf16),
            "wva": wva.astype(bf16),
            "wp": wp,
            "wpr": np.ascontiguousarray(w_proj[cols, :]).astype(bf16),
            "bq": np.ascontiguousarray(b_attn[cols]).reshape(P, 1),
            "bk": np.ascontiguousarray(
                b_attn[C + P * c: C + P * (c + 1)]).reshape(P, 1),
        })
    return in_maps


def kernel(x, w_attn, b_attn, w_proj, b_proj):
    from concourse.bass_utils import run_bass_kernel_spmd

    if "nc" not in _CACHED:
        _CACHED["nc"] = _build_nc()
    nc = _CACHED["nc"]

    in_maps = _prep_inputs(x, w_attn, b_attn, w_proj)
    res = run_bass_kernel_spmd(nc, in_maps, core_ids=list(range(NCORES)))

    # unit u (u = 2b + half) covers tokens [b*2048 + half*1024, +1024);
    # core c holds rows [+c*128, +128) of that range. Unit 7 comes back as
    # row-parallel partials summed here.
    y = np.empty((B, T, C), dtype=np.float32)
    for c in range(NCORES):
        part = res.results[c]["yp"].astype(np.float32)     # [6, 128, C]
        for u in range(6):
            b, half = u // 2, u % 2
            t0 = half * 1024 + c * 128
            y[b, t0:t0 + 128, :] = part[u]
    acc = res.results[0]["ypl"].astype(np.float32)
    for c in range(1, NCORES):
        acc += res.results[c]["ypl"].astype(np.float32)
    y[3, :, :] = acc
    # v-bias passes through attention unchanged (softmax weights sum to 1),
    # so it is folded into the projection bias here
    bv = np.asarray(b_attn, dtype=np.float32)[2 * C:]
    y += np.asarray(b_proj, dtype=np.float32) + \
        bv @ np.asarray(w_proj, dtype=np.float32)
    return y


# revision 27
# speedup vs baseline: 1.0073x; 1.0073x over previous
"""Causal self-attention (GPT-style block) on 8 Trainium2 NeuronCores.

Sharding: tensor-parallel over heads. 16 heads / 8 cores = 2 heads per core.
- c_attn column-parallel: each core computes q/k/v for its 2 heads from the
  full input x (streamed to every core).
- attention: fully local per core (its 2 heads, all 4 batches).
- c_proj token-parallel after an AllToAll of the attention output; each core
  returns fully-reduced output rows for its own token shard. The final
  half-batch skips the exchange (row-parallel partials summed on the host)
  so the tail never waits on a collective.

Implementation notes (all matmul operands bf16; rel-err budget is 2e-2):
- q,k are produced channel-major ([chan, tok]); v is produced TOKEN-major
  directly in stage 1 (stationary = x-tile, moving = w_v), with the v-bias
  and the softmax-denominator ones-column folded into a single K=1 matmul
  (moving row bva has bias values + 1.0 at the ones slots).
- Scores are computed transposed: S^T[key, query] with the 2 heads packed
  into the two 64-row halves of the PE array (tile_position row tiling).
- exp on ACT (scale=1/8) with one instruction per key tile (strided AP
  covers both heads); causal mask applied multiplicatively on ep (DVE).
- PV is computed in the [query, chan] orientation: stationary = ep tile
  [keys, 128 queries], moving = vaug [keys, 65] -> full 128-partition PE
  utilization, and the softmax denominator lands in psum column 64 as a
  per-partition scalar. Normalization is a single fused DVE multiply
  (psum * broadcast(1/l)) that also evicts to SBUF bf16.
- y (token-major) is exchanged per-batch via 3 AllToAlls + 1 single-unit
  AllToAll; receivers PE-transpose the 8 received [tok, chan] tiles to
  channel-major and run c_proj with the full w_proj.
- Emission is software-pipelined: stage-1 of block g+1 and ready
  projection units are interleaved ("filler chunks") inside the attention
  j-loops so the PE instruction stream never head-blocks on the ACT exp.
"""

import numpy as np

P = 128
B = 4
T = 2048
BT = B * T            # 8192 tokens
C = 1024
KT = C // P           # 8 contraction tiles of 128 input channels
NTB = BT // 512       # 16 token blocks of 512
HD = 64               # head dim
NQ = T // 512         # 4 query blocks per batch
NCORES = 8

_CACHED = {}


def _build_nc():
    import concourse.mybir as mybir
    import concourse.tile as tile
    from concourse import bacc
    from concourse.masks import make_identity

    f32 = mybir.dt.float32
    bf16 = mybir.dt.bfloat16
    EXP = mybir.ActivationFunctionType.Exp

    nc = bacc.Bacc("TRN2", target_bir_lowering=False, debug=False,
                   num_devices=NCORES)

    xp = nc.dram_tensor("xp", [NTB, P, KT, 512], bf16, kind="ExternalInput")
    wq = nc.dram_tensor("wq", [P, KT, P], bf16, kind="ExternalInput")
    wk = nc.dram_tensor("wk", [P, KT, P], bf16, kind="ExternalInput")
    wva = nc.dram_tensor("wva", [P, KT, P], bf16, kind="ExternalInput")
    wp = nc.dram_tensor("wp", [P, KT, C], bf16, kind="ExternalInput")
    wpr = nc.dram_tensor("wpr", [P, C], bf16, kind="ExternalInput")
    bq = nc.dram_tensor("bq", [P, 1], f32, kind="ExternalInput")
    bk = nc.dram_tensor("bk", [P, 1], f32, kind="ExternalInput")
    # exchange units 0..5: my 128 tokens x all 1024 channels, projected
    yp = nc.dram_tensor("yp", [6, P, C], bf16, kind="ExternalOutput")
    # units 6,7 (batch 3): row-parallel partials, host sums over cores
    ypl = nc.dram_tensor("ypl", [T, C], bf16, kind="ExternalOutput")

    with tile.TileContext(nc) as tc:
        with (
            tc.tile_pool(name="const", bufs=1) as const,
            tc.tile_pool(name="xt", bufs=2) as xt_pool,
            tc.tile_pool(name="slab", bufs=2) as slab_pool,
            tc.tile_pool(name="e", bufs=18) as e_pool,
            tc.tile_pool(name="r", bufs=4) as r_pool,
            tc.tile_pool(name="y", bufs=16) as y_pool,
            tc.tile_pool(name="ygt", bufs=2) as ygt_pool,
            tc.tile_pool(name="yg", bufs=2) as yg_pool,
            tc.tile_pool(name="ob", bufs=3) as ob_pool,
            tc.tile_pool(name="dram", bufs=1, space="DRAM") as dram_pool,
            tc.tile_pool(name="ps1", bufs=2, space="PSUM") as ps1_pool,
            tc.tile_pool(name="pss", bufs=2, space="PSUM") as pss_pool,
            tc.tile_pool(name="po", bufs=2, space="PSUM") as po_pool,
        ):
            # --- DRAM staging for the exchanges ---
            # per-batch AllToAll: dest core gets [2 units, 128 tok, 128 chan]
            g_in = [dram_pool.tile([NCORES, 2, P, P], bf16, name=f"g_in{b}",
                                   tag=f"g_in{b}") for b in range(3)]
            g_out = [dram_pool.tile([NCORES, 2, P, P], bf16, name=f"g_out{b}",
                                    tag=f"g_out{b}") for b in range(3)]

            # --- constants / weights resident in SBUF ---
            wq_sb = const.tile([P, KT, P], bf16)
            wk_sb = const.tile([P, KT, P], bf16)
            wva_sb = const.tile([P, KT, P], bf16)
            wp_sb = const.tile([P, KT, C], bf16)
            wpr_sb = const.tile([P, C], bf16)
            bq_sb = const.tile([P, 1], f32)
            bk_sb = const.tile([P, 1], f32)
            nc.sync.dma_start(wq_sb[:, 0], wq[:, 0])
            nc.sync.dma_start(wq_sb[:, 1:KT], wq[:, 1:KT])

            ident = const.tile([P, P], bf16)


            make_identity(nc, ident[:])
            # PE warmup: dummy matmuls on ident while the first DMAs land
            # ramp the p-state so real work starts at full clock
            wtmp = const.tile([P, P], bf16)
            nc.vector.memset(wtmp[:], 0.5)

            def warmup():
                pw = ps1_pool.tile([P, P], bf16, tag="ps1", name="warm")
                for _ in range(16):
                    nc.tensor.transpose(pw[:], wtmp[:], wtmp[:])

            # mask[p, s] = 1.0 if s >= p else 0.0 (keep upper-right triangle)
            mask_f = const.tile([P, P], f32)
            nc.gpsimd.memset(mask_f[:], 1.0)
            nc.gpsimd.affine_select(
                out=mask_f[:],
                in_=mask_f[:],
                compare_op=mybir.AluOpType.is_ge,
                fill=0.0,
                base=0,
                pattern=[[1, P]],
                channel_multiplier=-1,
            )
            mask_sb = const.tile([P, P], bf16)
            nc.vector.tensor_copy(mask_sb[:], mask_f[:])

            # ---------------- stage 1 (qkv) chunk emitters ----------------
            xt_tiles = {}

            def load_xt(g):
                xt_tiles[g] = xt_pool.tile([P, KT, 512], bf16, name=f"xt{g}", tag="xt")
                nc.sync.dma_start(xt_tiles[g][:], xp[g])

            # per-batch slabs, double-buffered (batch b and b+1 in flight)
            qT_t = {}
            kT_t = {}
            va_t = {}

            def get_slabs(b):
                if b not in qT_t:
                    qT_t[b] = slab_pool.tile([P, T], bf16, tag="qT",
                                             name=f"qT{b}")
                    kT_t[b] = slab_pool.tile([P, T], bf16, tag="kT",
                                             name=f"kT{b}")
                    va_t[b] = slab_pool.tile([P, 16, 2, 65], bf16, tag="va",
                                             name=f"va{b}")
                    nc.vector.memset(va_t[b][:, :, :, 64], 1.0)
                return qT_t[b], kT_t[b], va_t[b]

            def s1_chunks(b, lb):
                """Chunks for stage-1 of 512-token block lb of batch b."""
                g = b * NQ + lb
                qT, kT, vaug = get_slabs(b)
                sl = slice(lb * 512, (lb + 1) * 512)

                def qk(w_sb, b_sb, dst):
                    def run(w_sb=w_sb, b_sb=b_sb, dst=dst):
                        if g + 1 < NTB:
                            if g + 1 not in xt_tiles:
                                load_xt(g + 1)
                        xt = xt_tiles[g]
                        ps = ps1_pool.tile([P, 512], f32, tag="ps1", name="ps")
                        for kt in range(KT):
                            nc.tensor.matmul(ps[:], w_sb[:, kt, :],
                                             xt[:, kt, :],
                                             start=(kt == 0),
                                             stop=(kt == KT - 1))
                        nc.vector.tensor_scalar_add(dst[:, sl], ps[:], b_sb[:])
                    return run

                def vpair(half):
                    def run(half=half):
                        xt = xt_tiles[g]
                        psv = ps1_pool.tile([P, 2, 2, 64], f32, tag="ps1", name="psv")
                        for ts2 in range(2):
                            ts = half * 2 + ts2
                            tsl = slice(ts * P, (ts + 1) * P)
                            for kt in range(KT):
                                nc.tensor.matmul(psv[:, ts2], xt[:, kt, tsl],
                                                 wva_sb[:, kt],
                                                 start=(kt == 0),
                                                 stop=(kt == KT - 1))
                        jt = lb * 4 + half * 2
                        nc.vector.tensor_copy(vaug[:, jt:jt + 2, :, 0:64],
                                              psv[:])
                    return run

                cks = [qk(wq_sb, bq_sb, qT), qk(wk_sb, bk_sb, kT),
                       vpair(0), vpair(1)]
                for ck, cost in zip(cks, (1900, 1800, 1100, 1100)):
                    ck.cost = cost
                return cks

            # ---------------- projection chunk emitters ----------------
            wp_loaded = []

            def load_wp():
                if not wp_loaded:
                    nc.sync.dma_start(wp_sb[:], wp[:])
                    nc.sync.dma_start(wpr_sb[:], wpr[:])
                    wp_loaded.append(True)

            def proj_chunks(u):
                """Project exchange unit u (0..6): my 128 tokens x 1024 ch."""
                state = {}

                def load_tr(half):
                    def run(half=half):
                        if half == 0:
                            yg = yg_pool.tile([P, NCORES, P], bf16, tag="yg", name="yg")
                            nc.sync.dma_start(
                                yg[:],
                                g_out[u // 2][:, u % 2].rearrange(
                                    "s p t -> p s t"))
                            state["yg"] = yg
                            state["ygT"] = ygt_pool.tile([P, NCORES, P], bf16,
                                                         tag="ygt", name="ygT")
                        yg, ygT = state["yg"], state["ygT"]
                        pst = po_pool.tile([P, 512], bf16, tag="po",
                                           name="pst")
                        for s in range(4):
                            src = half * 4 + s
                            nc.tensor.transpose(pst[:, s * P:(s + 1) * P],
                                                yg[:, src, :], ident[:])
                        nc.vector.tensor_copy(
                            ygT[:, half * 4:(half + 1) * 4], pst[:])
                    return run

                def mm(half):
                    def run(half=half):
                        ygT = state["ygT"]
                        pp = ps1_pool.tile([P, 512], f32, tag="ps1", name="pp")
                        for ct in range(KT):
                            nc.tensor.matmul(
                                pp[:], ygT[:, ct, :],
                                wp_sb[:, ct, half * 512:(half + 1) * 512],
                                start=(ct == 0), stop=(ct == KT - 1))
                        if half == 0:
                            state["ob"] = ob_pool.tile([P, C], bf16, tag="ob", name="ob")
                        ob = state["ob"]
                        # DVE for both halves: ACT's in-order queue is
                        # exp-latency-critical in these windows
                        if half == 0:
                            nc.vector.tensor_copy(ob[:, 0:512], pp[:])
                        else:
                            nc.vector.tensor_copy(ob[:, 512:C], pp[:])
                            nc.sync.dma_start(yp[u], ob[:])
                    return run

                cks = [load_tr(0), load_tr(1), mm(0), mm(1)]
                for ck, cost in zip(cks, (400, 300, 1750, 1750)):
                    ck.cost = cost
                return cks

            def partial_chunks(u, tiles=range(8)):
                """Row-parallel partial projection of b3 half-batch u-6,
                pipelined: transposes run ahead of the matmuls so the PE
                never waits a full DVE eviction round-trip."""
                i0 = (u - 6) * 2
                yT = {}

                def tr(t):
                    def run(t=t):
                        ytile = y_tiles[(3, i0 + t // 4)][t % 4]
                        pst = ps1_pool.tile([P, P], bf16, tag="ps1",
                                            name="pstp")
                        nc.tensor.transpose(pst[:], ytile[:], ident[:])
                        yT[t] = ygt_pool.tile([P, P], bf16, tag="ygt",
                                              name="yTp")
                        nc.vector.tensor_copy(yT[t][:], pst[:])
                    return run

                def mm(t):
                    def run(t=t):
                        row = (u - 6) * 1024 + t * P
                        pp0 = ps1_pool.tile([P, 512], f32, tag="ps1",
                                            name="pp0")
                        nc.tensor.matmul(pp0[:], yT[t][:], wpr_sb[:, 0:512],
                                         start=True, stop=True)
                        ob = ob_pool.tile([P, C], bf16, tag="ob", name="obp")
                        nc.vector.tensor_copy(ob[:, 0:512], pp0[:])
                        pp1 = ps1_pool.tile([P, 512], f32, tag="ps1",
                                            name="pp1")
                        nc.tensor.matmul(pp1[:], yT[t][:], wpr_sb[:, 512:C],
                                         start=True, stop=True)
                        nc.scalar.copy(ob[:, 512:C], pp1[:])
                        nc.sync.dma_start(ypl[row:row + P, :], ob[:])
                    return run

                if tiles is None:
                    return tr, mm
                tl = list(tiles)
                out = [tr(t) for t in tl[:2]]
                for k, t in enumerate(tl):
                    if k + 2 < len(tl):
                        out.append(tr(tl[k + 2]))
                    out.append(mm(t))
                return out

            # ---------------- attention ----------------
            y_tiles = {}   # (b, i) -> [y tile per subtile gt2]
            pending_eps = {}   # (b, i) -> {j: ep} pre-emitted score tiles

            def emit_s(b, i, j):
                qT, kT, vaug = get_slabs(b)
                qsl0 = i * 512
                q0 = max(0, j - 4 * i) * P
                jsl = slice(j * P, (j + 1) * P)
                qsl = slice(qsl0 + q0, qsl0 + 512)
                psp = pss_pool.tile([P, 2, 512], f32, tag="pss",
                                    name=f"psp{j % 2}")
                nc.tensor.matmul(psp[:, 0, q0:512], kT[0:HD, jsl],
                                 qT[0:HD, qsl], start=True, stop=True,
                                 tile_position=(0, 0))
                nc.tensor.matmul(psp[:, 1, q0:512], kT[HD:P, jsl],
                                 qT[HD:P, qsl], start=True, stop=True,
                                 tile_position=(HD, 0))
                ep = e_pool.tile([P, 2, 512], bf16, tag="e",
                                 name=f"ep{j}")
                nc.scalar.activation(ep[:, :, q0:512], psp[:, :, q0:512],
                                     EXP, scale=0.125)
                if j - 4 * i >= 0:
                    msl = slice(q0, q0 + P)
                    nc.gpsimd.tensor_mul(
                        ep[:, :, msl], ep[:, :, msl],
                        mask_sb[:].unsqueeze(1).broadcast_to((P, 2, P)))
                return ep

            def att_block(b, i, filler, post_gt=None, next_blk=None):
                """Query block i of batch b, pumping filler chunks inside.

                Two phases per block: an S-phase (scores+exp+mask, all ep
                tiles kept resident) and a PV-phase where each (subtile,
                head) PSUM accumulation group runs CONSECUTIVELY — real HW
                corrupts interleaved open groups within one psum bank.
                The next block's first two score tiles are pre-emitted
                before the PV-phase so the ACT exp stream never starves.
                """
                qT, kT, vaug = get_slabs(b)
                nj = 4 * (i + 1)

                # --- S-phase ---
                eps = pending_eps.pop((b, i), {})
                fi = 0
                for j in range(nj):
                    if j not in eps:
                        eps[j] = emit_s(b, i, j)
                    while fi * nj < (j + 1) * len(filler):
                        filler[fi]()
                        fi += 1
                if next_blk is not None:
                    nb, ni = next_blk
                    pending_eps[(nb, ni)] = {
                        j: emit_s(nb, ni, j) for j in range(2)}

                # --- PV-phase ---
                ytl = []
                for gt2 in range(4):
                    nd = 4 * i + gt2 + 1   # visible key tiles
                    po = po_pool.tile([P, 2, 65], f32, tag="po", name="po")
                    for h in range(2):
                        for j in range(nd):
                            nc.tensor.matmul(
                                po[:, h], eps[j][:, h, gt2 * P:(gt2 + 1) * P],
                                vaug[:, j, h, :], start=(j == 0),
                                stop=(j == nd - 1))
                    # fused normalize + eviction: y = po * broadcast(1/l)
                    ytile = y_pool.tile([P, 2, 64], bf16, tag="y", name="yt")
                    ytl.append(ytile)
                    r = r_pool.tile([P, 2], f32, tag="r", name="r")
                    nc.vector.reciprocal(r[:], po[:, :, 64])
                    nc.vector.tensor_mul(
                        ytile[:], po[:, :, 0:64],
                        r[:].unsqueeze(2).broadcast_to((P, 2, 64)))
                    u = 2 * b + i // 2
                    if u < 6:
                        # A2A staging (dest core = subtile index in half)
                        dst = (i % 2) * 4 + gt2
                        nc.sync.dma_start(g_in[b][dst, i // 2], ytile[:])
                    # u >= 6: consumed locally by partial_chunks
                    y_tiles[(b, i)] = ytl
                    if post_gt is not None:
                        for f in post_gt(gt2):
                            f()
                for f in filler[fi:]:
                    f()

            # ---------------- schedule ----------------
            def exchange(b):
                nc.gpsimd.collective_compute(
                    "AllToAll",
                    mybir.AluOpType.bypass,
                    replica_groups=[list(range(NCORES))],
                    ins=[g_in[b][:]],
                    outs=[g_out[b][:]],
                )

            # first x block arrives per-kt so the first q matmuls can
            # start as soon as wq + kt0 land
            xt_tiles[0] = xt_pool.tile([P, KT, 512], bf16, name="xt0",
                                       tag="xt")
            warmup()
            nc.sync.dma_start(xt_tiles[0][:, 0:4], xp[0][:, 0:4])
            nc.sync.dma_start(wk_sb[:], wk[:])
            nc.sync.dma_start(xt_tiles[0][:, 4:KT], xp[0][:, 4:KT])
            nc.sync.dma_start(bq_sb[:], bq[:])
            nc.sync.dma_start(bk_sb[:], bk[:])
            nc.sync.dma_start(wva_sb[:], wva[:])
            for f in s1_chunks(0, 0):
                f()

            # filler assignment per attention window (b, i)
            windows = {}
            for b in range(B):
                for i in range(NQ):
                    g = b * NQ + i + 1   # next stage-1 block, pipelined
                    windows[(b, i)] = s1_chunks(g // NQ, g % NQ) \
                        if g < NTB else []
            # projection units placed once their collective has finished
            PROJ_SLOT = {0: (2, 0), 1: (2, 1), 2: (2, 3), 3: (3, 0),
                         4: (3, 3), 5: (3, 3)}
            proj_lists = {}
            for u, slot in PROJ_SLOT.items():
                proj_lists.setdefault(slot, []).append(u)
            p6tr, p6mm = partial_chunks(6, None)
            p7tr, p7mm = partial_chunks(7, None)

            for b in range(B):
                for i in range(NQ):
                    filler = list(windows[(b, i)])
                    for u in proj_lists.get((b, i), []):
                        filler += proj_chunks(u)
                    post_gt = None
                    if (b, i) == (3, 2):
                        filler += partial_chunks(6)
                    if (b, i) == (3, 3):
                        filler += partial_chunks(7, range(4))

                        def post_gt(gt2):
                            # drain U7 tiles 4..7 inline with the PV-phase
                            out = [p7tr(4 + gt2)]
                            if gt2 > 0:
                                out.append(p7mm(3 + gt2))
                            return out
                    if (b, i) == (1, 2):
                        load_wp()
                    nxt = (b, i + 1) if i < 3 else \
                        ((b + 1, 0) if b < 3 else None)
                    att_block(b, i, filler, post_gt, nxt)
                    if i == 3 and b < 3:
                        exchange(b)

            p7mm(7)()

    nc.compile()
    return nc


def _prep_inputs(x, w_attn, b_attn, w_proj):
    import ml_dtypes
    bf16 = ml_dtypes.bfloat16

    x = np.asarray(x, dtype=np.float32)
    w_attn = np.asarray(w_attn, dtype=np.float32)
    b_attn = np.asarray(b_attn, dtype=np.float32)
    w_proj = np.asarray(w_proj, dtype=np.float32)

    x_flat = x.reshape(BT, C)
    # xp[tb, p, kt, s] = x_flat[tb*512+s, kt*128+p]
    xp = np.ascontiguousarray(
        x_flat.T.reshape(KT, P, NTB, 512).transpose(2, 1, 0, 3)).astype(bf16)

    wp = np.ascontiguousarray(
        w_proj.reshape(KT, P, C).transpose(1, 0, 2)).astype(bf16)
    in_maps = []
    for c in range(NCORES):
        cols = slice(P * c, P * (c + 1))

        def wslice(off):
            w = w_attn[:, off + P * c: off + P * (c + 1)]   # [1024, 128]
            return np.ascontiguousarray(
                w.reshape(KT, P, P).transpose(1, 0, 2))

        wva = wslice(2 * C)                                  # [P, KT, 128]
        in_maps.append({
            "xp": xp,
            "wq": wslice(0).astype(bf16),
            "wk": wslice(C).astype(bf16),
            "wva": wva.astype(bf16),
            "wp": wp,
            "wpr": np.ascontiguousarray(w_proj[cols, :]).astype(bf16),
            "bq": np.ascontiguousarray(b_attn[cols]).reshape(P, 1),
            "bk": np.ascontiguousarray(
                b_attn[C + P * c: C + P * (c + 1)]).reshape(P, 1),
        })
    return in_maps


def kernel(x, w_attn, b_attn, w_proj, b_proj):
    from concourse.bass_utils import run_bass_kernel_spmd

    if "nc" not in _CACHED:
        _CACHED["nc"] = _build_nc()
    nc = _CACHED["nc"]

    in_maps = _prep_inputs(x, w_attn, b_attn, w_proj)
    res = run_bass_kernel_spmd(nc, in_maps, core_ids=list(range(NCORES)))

    # unit u (u = 2b + half) covers tokens [b*2048 + half*1024, +1024);
    # core c holds rows [+c*128, +128) of that range. Unit 7 comes back as
    # row-parallel partials summed here.
    y = np.empty((B, T, C), dtype=np.float32)
    for c in range(NCORES):
        part = res.results[c]["yp"].astype(np.float32)     # [6, 128, C]
        for u in range(6):
            b, half = u // 2, u % 2
            t0 = half * 1024 + c * 128
            y[b, t0:t0 + 128, :] = part[u]
    acc = res.results[0]["ypl"].astype(np.float32)
    for c in range(1, NCORES):
        acc += res.results[c]["ypl"].astype(np.float32)
    y[3, :, :] = acc
    # v-bias passes through attention unchanged (softmax weights sum to 1),
    # so it is folded into the projection bias here
    bv = np.asarray(b_attn, dtype=np.float32)[2 * C:]
    y += np.asarray(b_proj, dtype=np.float32) + \
        bv @ np.asarray(w_proj, dtype=np.float32)
    return y
